# revision 1
# baseline (speedup 1.0000x reference)
"""MultiEdgeGraphBlock kernel for 8 Trainium2 NeuronCores — v3.

v2 design (node-8th sharding, 4-batch-fused 2048B gather rows, single SWDGE
queue) plus pipeline fixes:
  - software pipeline: LN/MLP for block k-1 is emitted between block k's
    aggregation and block k+1's gathers, so PE-FIFO order never stalls the
    gather side on the previous block's MLP tail.
  - LN stats are broadcast across partitions by using an all-ones [128,128]
    stationary in the stats matmuls (every partition gets the column sum),
    eliminating the per-block DRAM round-trip for mu/rstd.
  - all gather-index tiles are preloaded into SBUF at startup (12.5 KB/part),
    so gathers never wait on per-block HWDGE index loads.
  - hT loads go via the ACT HWDGE queue, decoupled from outT stores on SP.
"""

import sys

sys.path.insert(0, "/opt/trn_rl_repo")

import numpy as np
import ml_dtypes
from contextlib import ExitStack

import concourse.bass as bass
import concourse.mybir as mybir
import concourse.tile as tile
from concourse import bacc
from concourse.bass_utils import run_bass_kernel_spmd

BF16 = ml_dtypes.bfloat16
F32 = mybir.dt.float32
BF = mybir.dt.bfloat16
I16 = mybir.dt.int16
AO = mybir.AluOpType
AF = mybir.ActivationFunctionType

B, N, F, E, DEG, H = 4, 10000, 256, 5, 16, 256
NLOC = N // 8          # 1250 nodes per core
NBLK = 10              # 128-node blocks per core (1280 padded)
NPADC = NBLK * 128     # 1280
BLK = 512              # LN/MLP block = 4 batches x 128 nodes
VCOLS = NBLK * BLK     # 5120 virtual columns per core
ZROW = N               # zero-row index in gather table
GIDX = DEG * 128       # 2048 indices per gather call
ROWE = B * F           # 1024 elems per fused row
NQUEUES = 1
LN_EPS = 1e-6

_PROGRAM = {}
VARIANT = "full"  # full | gather | nogather
# build stages for bisection: 1=gather+reduce, 2=+mean, 3=+transpose,
# 4=+Wmatmul+aggevict, 5=full (LN/MLP/stores)
STAGE = 5


def _build_program(repeat=1):
    nc = bacc.Bacc(
        "TRN2",
        target_bir_lowering=False,
        debug=False,
        dynamic_dma_scratch_size=32768,
        num_swdge_queues=NQUEUES,
    )

    tbl4 = nc.dram_tensor("tbl4", [N + 1, ROWE], BF, kind="ExternalInput")
    hT = nc.dram_tensor("hT", [F, VCOLS], F32, kind="ExternalInput")
    idxw = nc.dram_tensor("idxw", [128, E * NBLK * 128], I16, kind="ExternalInput")
    masknm = nc.dram_tensor("masknm", [128, E, NBLK, DEG], F32, kind="ExternalInput")
    w_pe = nc.dram_tensor("w_pe", [128, E * 4, 128], BF, kind="ExternalInput")
    w1 = nc.dram_tensor("w1", [128, 8, 128], BF, kind="ExternalInput")
    w2 = nc.dram_tensor("w2", [128, 4, 128], BF, kind="ExternalInput")
    ident_d = nc.dram_tensor("ident", [128, 128], BF, kind="ExternalInput")
    onesf_d = nc.dram_tensor("onesf", [128, 128], BF, kind="ExternalInput")
    b1_d = nc.dram_tensor("b1pc", [128, 2], F32, kind="ExternalInput")
    b2_d = nc.dram_tensor("b2pc", [128, 2], F32, kind="ExternalInput")
    lns_d = nc.dram_tensor("lnspc", [128, 4], F32, kind="ExternalInput")
    lnb_d = nc.dram_tensor("lnbpc", [128, 4], F32, kind="ExternalInput")
    bedg_d = nc.dram_tensor("bedgpc", [128, 2, E], F32, kind="ExternalInput")

    outT = nc.dram_tensor("outT", [F, VCOLS], F32, kind="ExternalOutput")

    with tile.TileContext(nc) as tc, ExitStack() as ctx:
        cpool = ctx.enter_context(tc.tile_pool(name="const", bufs=1))
        spsum = ctx.enter_context(tc.tile_pool(name="spsum", bufs=2, space="PSUM"))
        mtpsum = ctx.enter_context(tc.tile_pool(name="mtpsum", bufs=2, space="PSUM"))
        apsum = ctx.enter_context(tc.tile_pool(name="apsum", bufs=1, space="PSUM"))
        gpool = ctx.enter_context(tc.tile_pool(name="g", bufs=2))
        xpool = ctx.enter_context(tc.tile_pool(name="x", bufs=2))
        wpool = ctx.enter_context(tc.tile_pool(name="work", bufs=2))

        # ---------------- constants ----------------
        W_sb = cpool.tile([128, E * 4, 128], BF)
        nc.sync.dma_start(W_sb[:], w_pe[:])
        W1_sb = cpool.tile([128, 8, 128], BF)
        nc.sync.dma_start(W1_sb[:], w1[:])
        W2_sb = cpool.tile([128, 4, 128], BF)
        nc.sync.dma_start(W2_sb[:], w2[:])
        id_sb = cpool.tile([128, 128], BF)
        nc.sync.dma_start(id_sb[:], ident_d[:])
        of_sb = cpool.tile([128, 128], BF)
        nc.sync.dma_start(of_sb[:], onesf_d[:])
        b1_sb = cpool.tile([128, 2], F32)
        nc.sync.dma_start(b1_sb[:], b1_d[:])
        b2_sb = cpool.tile([128, 2], F32)
        nc.sync.dma_start(b2_sb[:], b2_d[:])
        lns_sb = cpool.tile([128, 4], F32)
        nc.sync.dma_start(lns_sb[:], lns_d[:])
        lnb_sb = cpool.tile([128, 4], F32)
        nc.sync.dma_start(lnb_sb[:], lnb_d[:])
        bedg_sb = cpool.tile([128, 2, E], F32)
        nc.sync.dma_start(bedg_sb[:], bedg_d[:])
        bsum_sb = cpool.tile([128, 2], F32)
        nc.vector.tensor_reduce(
            bsum_sb[:], bedg_sb[:], axis=mybir.AxisListType.X, op=AO.add
        )

        # all gather indices, resident in SBUF for the whole kernel
        idx_sb = cpool.tile([128, E * NBLK * 128], I16)
        nc.sync.dma_start(idx_sb[:], idxw[:])

        # ---------------- reciprocal degree (node-major) ----------------
        mask_sb = cpool.tile([128, E, NBLK, DEG], F32)
        nc.sync.dma_start(mask_sb[:], masknm[:])
        dn_sb = cpool.tile([128, E, NBLK], F32)
        for i in range(E):
            nc.vector.tensor_reduce(
                dn_sb[:, i, :], mask_sb[:, i], axis=mybir.AxisListType.X, op=AO.add
            )
        nc.vector.tensor_scalar_max(dn_sb[:], dn_sb[:], 1.0)
        recip_sb = cpool.tile([128, E, NBLK], F32)
        nc.vector.reciprocal(recip_sb[:], dn_sb[:])

        def emit_gather_block(kk):
            """Gathers + masked-mean + aggregation for node block kk.
            Returns the assembled x tile [128, 4, BLK]."""
            ns = bass.ts(kk, BLK)
            x = xpool.tile([128, 4, BLK], F32)
            nc.scalar.dma_start(x[:, 0, :], hT[0:128, ns])
            nc.scalar.dma_start(x[:, 1, :], hT[128:256, ns])

            agg = apsum.tile([128, 2, BLK], F32, tag="ps")
            for i in range(E):
                G = gpool.tile([128, DEG, ROWE], BF, tag="G")
                ga = G[:]
                gap = bass.AP(
                    ga.tensor, ga.offset, [ga.ap[0], [ROWE, DEG], [1, ROWE]]
                )
                if VARIANT != "nogather":
                    nc.gpsimd.dma_gather(
                        out_ap=gap,
                        in_ap=tbl4.ap(),
                        idxs_ap=idx_sb[:, (i * NBLK + kk) * 128 : (i * NBLK + kk + 1) * 128],
                        num_idxs=GIDX,
                        num_idxs_reg=GIDX,
                        elem_size=ROWE,
                        single_packet=False,
                        queue_num=0,
                    )
                S = spsum.tile([128, 2, BLK], F32, tag="S")
                ND = 1 if VARIANT == "gather" else DEG
                for hh in range(2):
                    for d in range(ND):
                        nc.tensor.matmul(
                            S[:, hh, :],
                            id_sb[:],
                            G[:, d, hh * BLK : (hh + 1) * BLK],
                            start=(d == 0),
                            stop=(d == ND - 1),
                        )
                if VARIANT == "gather" or STAGE < 2:
                    continue
                # masked mean: per-partition (node) reciprocal scale
                mean = wpool.tile([128, 2, BLK], BF, tag="mean")
                for hh in range(2):
                    nc.scalar.activation(
                        mean[:, hh, :], S[:, hh, :], AF.Copy,
                        scale=recip_sb[:, i, kk : kk + 1],
                    )
                if STAGE < 3:
                    continue
                # transpose to feature-major; mean cols: b*256 + c*128 + f
                mT = mtpsum.tile([128, 8, 128], BF, tag="mT")
                mv = mean[:]
                mflat = bass.AP(mv.tensor, mv.offset, [mv.ap[0], [1, 2 * BLK]])
                for bc in range(8):
                    nc.tensor.transpose(
                        mT[:, bc, :],
                        mflat[:, bc * 128 : (bc + 1) * 128],
                        id_sb[:],
                    )
                mT_sb = wpool.tile([128, 8, 128], BF, tag="mTsb")
                nc.scalar.copy(mT_sb[:], mT[:])
                if STAGE < 4:
                    continue
                # aggregated[f, m, b*128+n] += W_i[c,m]^T @ meanT[b,c]
                # (exactly one start=True per PSUM bank: i==0, b==0, c==0)
                for m in range(2):
                    for b in range(4):
                        for c in range(2):
                            nc.tensor.matmul(
                                agg[:, m, b * 128 : (b + 1) * 128],
                                W_sb[:, (i * 2 + c) * 2 + m, :],
                                mT_sb[:, b * 2 + c, :],
                                start=(i == 0 and b == 0 and c == 0),
                                stop=(i == E - 1 and b == 3 and c == 1),
                            )
            # aggregated -> x bottom half (+ sum of edge biases)
            for m in range(2 if (VARIANT != "gather" and STAGE >= 4) else 0):
                nc.scalar.activation(
                    x[:, 2 + m, :],
                    agg[:, m, :],
                    AF.Identity,
                    bias=bsum_sb[:, m : m + 1],
                    scale=1.0,
                )
            return x

        def emit_ln_mlp(kk, x):
            """LayerNorm + MLP + residual + store for block kk given x."""
            ns = bass.ts(kk, BLK)
            st = apsum.tile([128, 2, BLK], F32, tag="ps")
            xbs = []
            for c in range(4):
                xb = wpool.tile([128, BLK], BF, tag=f"xb{c}")
                nc.vector.tensor_copy(xb[:], x[:, c, :])
                xbs.append(xb)
                nc.tensor.matmul(
                    st[:, 0, :], of_sb[:], xb[:],
                    start=(c == 0), stop=(c == 3),
                )
            for c in range(4):
                xsq = wpool.tile([128, BLK], BF, tag="xsq")
                nc.scalar.square(xsq[:], xbs[c][:])
                nc.tensor.matmul(
                    st[:, 1, :], of_sb[:], xsq[:],
                    start=(c == 0), stop=(c == 3),
                )
            # mu/rstd, broadcast across partitions already (ones stationary)
            mu_t = wpool.tile([128, BLK], F32, tag="mu")
            nc.vector.tensor_scalar_mul(mu_t[:], st[:, 0, :], 1.0 / 512.0)
            mu2 = wpool.tile([128, BLK], F32, tag="mu2")
            nc.vector.tensor_mul(mu2[:], mu_t[:], mu_t[:])
            nc.vector.tensor_scalar_sub(mu2[:], mu2[:], LN_EPS)
            var = wpool.tile([128, BLK], F32, tag="var")
            nc.vector.scalar_tensor_tensor(
                var[:], st[:, 1, :], 1.0 / 512.0, mu2[:],
                op0=AO.mult, op1=AO.subtract,
            )
            sd = wpool.tile([128, BLK], F32, tag="sd")
            nc.scalar.activation(sd[:], var[:], AF.Sqrt, bias=0.0)
            rstd = wpool.tile([128, BLK], F32, tag="rstd")
            nc.vector.reciprocal(rstd[:], sd[:])

            xln = wpool.tile([128, 4, BLK], BF, tag="xln")
            for c in range(4):
                tt = wpool.tile([128, BLK], F32, tag="tt")
                nc.vector.scalar_tensor_tensor(
                    tt[:], x[:, c, :], 0.0, mu_t[:],
                    op0=AO.add, op1=AO.subtract,
                )
                nc.vector.tensor_mul(tt[:], tt[:], rstd[:])
                nc.scalar.activation(
                    xln[:, c, :], tt[:], AF.Identity,
                    bias=lnb_sb[:, c : c + 1], scale=lns_sb[:, c : c + 1],
                )

            # ---------------- MLP ----------------
            y1 = apsum.tile([128, 2, BLK], F32, tag="ps")
            for m in range(2):
                for k in range(4):
                    nc.tensor.matmul(
                        y1[:, m, :], W1_sb[:, k * 2 + m, :], xln[:, k, :],
                        start=(k == 0), stop=(k == 3),
                    )
            y1b = wpool.tile([128, 2, BLK], BF, tag="y1b")
            for m in range(2):
                nc.scalar.activation(
                    y1b[:, m, :], y1[:, m, :], AF.Relu,
                    bias=b1_sb[:, m : m + 1], scale=1.0,
                )
            y2 = apsum.tile([128, 2, BLK], F32, tag="ps")
            for m in range(2):
                for k in range(2):
                    nc.tensor.matmul(
                        y2[:, m, :], W2_sb[:, k * 2 + m, :], y1b[:, k, :],
                        start=(k == 0), stop=(k == 1),
                    )
            ot = wpool.tile([128, 2, BLK], F32, tag="ot")
            for m in range(2):
                nc.vector.scalar_tensor_tensor(
                    ot[:, m, :], y2[:, m, :], b2_sb[:, m : m + 1], x[:, m, :],
                    op0=AO.add, op1=AO.add,
                )
            for m in range(2):
                nc.sync.dma_start(outT[m * 128 : (m + 1) * 128, ns], ot[:, m, :])

        # ---------------- software-pipelined main loop ----------------
        do_tail = VARIANT != "gather" and STAGE >= 5
        for rep in range(repeat):
            prev = None  # (kk, x)
            for kk in range(NBLK):
                x = emit_gather_block(kk)
                if prev is not None and do_tail:
                    emit_ln_mlp(*prev)
                prev = (kk, x)
            if do_tail:
                emit_ln_mlp(*prev)

    nc.compile()
    return nc


def _get_program(repeat=1):
    key = (repeat, VARIANT)
    if key not in _PROGRAM:
        _PROGRAM[key] = _build_program(repeat)
    return _PROGRAM[key]


def _prep_shared(h, edge_indices, edge_masks, W_edge, b_edge, ln_scale, ln_bias,
                 W1, b1, W2, b2):
    """Host-side layout prep: weights/constants (shared) + per-core indices."""
    W_pe = np.empty((128, E * 4, 128), np.float32)
    for i in range(E):
        for c in range(2):
            for m in range(2):
                W_pe[:, (i * 2 + c) * 2 + m, :] = W_edge[
                    i, c * 128 : (c + 1) * 128, m * 128 : (m + 1) * 128
                ]
    W1b = np.empty((128, 8, 128), np.float32)
    for k in range(4):
        for m in range(2):
            W1b[:, k * 2 + m, :] = W1[k * 128 : (k + 1) * 128, m * 128 : (m + 1) * 128]
    W2b = np.empty((128, 4, 128), np.float32)
    for k in range(2):
        for m in range(2):
            W2b[:, k * 2 + m, :] = W2[k * 128 : (k + 1) * 128, m * 128 : (m + 1) * 128]

    # 4-batch fused gather table: tbl4[v, b*256+f] = h[b, v, f]
    tbl4 = np.zeros((N + 1, ROWE), BF16)
    tbl4[:N] = np.ascontiguousarray(
        h.transpose(1, 0, 2).reshape(N, ROWE)
    ).astype(BF16)

    shared = dict(
        tbl4=tbl4,
        w_pe=W_pe.astype(BF16),
        w1=W1b.astype(BF16),
        w2=W2b.astype(BF16),
        ident=np.eye(128, dtype=BF16),
        onesf=np.ones((128, 128), BF16),
        b1pc=np.ascontiguousarray(b1.reshape(2, 128).T.astype(np.float32)),
        b2pc=np.ascontiguousarray(b2.reshape(2, 128).T.astype(np.float32)),
        lnspc=np.ascontiguousarray(ln_scale.reshape(4, 128).T.astype(np.float32)),
        lnbpc=np.ascontiguousarray(ln_bias.reshape(4, 128).T.astype(np.float32)),
        bedgpc=np.ascontiguousarray(
            b_edge.T.reshape(2, 128, E).transpose(1, 0, 2).astype(np.float32)
        ),
    )

    cores = []
    for core in range(8):
        n0 = core * NLOC
        idx = edge_indices[:, n0 : n0 + NLOC, :].astype(np.int64)  # [E, NLOC, DEG]
        msk = edge_masks[:, n0 : n0 + NLOC, :]
        idx = np.where(idx < 0, 0, idx)
        sel = np.where(msk > 0, idx, ZROW).astype(np.int32)
        sel = np.concatenate(
            [sel, np.full((E, NPADC - NLOC, DEG), ZROW, np.int32)], axis=1
        )  # [E, NPADC, DEG]
        selT = sel.transpose(0, 2, 1)  # [E, DEG, NPADC]
        blocks = selT.reshape(E, DEG, NBLK, 128).transpose(0, 2, 1, 3)
        L = blocks.reshape(E, NBLK, GIDX)  # j = d*128 + n
        Wv = L.reshape(E, NBLK, GIDX // 16, 16).transpose(0, 1, 3, 2)
        idxw = np.tile(Wv, (1, 1, 8, 1)).astype(np.int16)  # [E, NBLK, 128, 128]
        # SBUF-resident layout: [128, E*NBLK*128]
        idxw = np.ascontiguousarray(
            idxw.transpose(2, 0, 1, 3).reshape(128, E * NBLK * 128)
        )

        mpad = np.concatenate(
            [msk, np.zeros((E, NPADC - NLOC, DEG), np.float32)], axis=1
        )  # [E, NPADC, DEG]
        masknm = np.ascontiguousarray(
            mpad.reshape(E, NBLK, 128, DEG).transpose(2, 0, 1, 3).astype(np.float32)
        )  # [128, E, NBLK, DEG]

        # hT[f, kk*512 + b*128 + n] = h[b, n0 + kk*128 + n, f]
        hp = np.zeros((B, NPADC, F), np.float32)
        hp[:, :NLOC] = h[:, n0 : n0 + NLOC, :]
        hTl = np.ascontiguousarray(
            hp.reshape(B, NBLK, 128, F).transpose(3, 1, 0, 2).reshape(F, VCOLS)
        )
        m = dict(hT=hTl, idxw=idxw, masknm=masknm)
        m.update(shared)
        cores.append(m)
    return cores


def kernel(**inputs):
    h = np.asarray(inputs["h"], np.float32)
    nc = _get_program()
    in_maps = _prep_shared(
        h,
        np.asarray(inputs["edge_indices"]),
        np.asarray(inputs["edge_masks"], np.float32),
        np.asarray(inputs["W_edge"], np.float32),
        np.asarray(inputs["b_edge"], np.float32),
        np.asarray(inputs["ln_scale"], np.float32),
        np.asarray(inputs["ln_bias"], np.float32),
        np.asarray(inputs["W1"], np.float32),
        np.asarray(inputs["b1"], np.float32),
        np.asarray(inputs["W2"], np.float32),
        np.asarray(inputs["b2"], np.float32),
    )

    res = run_bass_kernel_spmd(nc, in_maps, core_ids=list(range(8)))

    out = np.empty((B, N, F), np.float32)
    for core in range(8):
        n0 = core * NLOC
        o = res.results[core]["outT"]  # [F, VCOLS]
        ob = o.reshape(F, NBLK, B, 128).transpose(2, 1, 3, 0).reshape(B, NPADC, F)
        out[:, n0 : n0 + NLOC, :] = ob[:, :NLOC]
    return out



# revision 4
# speedup vs baseline: 2.2830x; 2.2830x over previous
"""MultiEdgeGraphBlock kernel for 8 Trainium2 NeuronCores — v3.

v2 design (node-8th sharding, 4-batch-fused 2048B gather rows, single SWDGE
queue) plus pipeline fixes:
  - software pipeline: LN/MLP for block k-1 is emitted between block k's
    aggregation and block k+1's gathers, so PE-FIFO order never stalls the
    gather side on the previous block's MLP tail.
  - LN stats are broadcast across partitions by using an all-ones [128,128]
    stationary in the stats matmuls (every partition gets the column sum),
    eliminating the per-block DRAM round-trip for mu/rstd.
  - all gather-index tiles are preloaded into SBUF at startup (12.5 KB/part),
    so gathers never wait on per-block HWDGE index loads.
  - hT loads go via the ACT HWDGE queue, decoupled from outT stores on SP.
"""

import os
import sys

sys.path.insert(0, "/opt/trn_rl_repo")

import numpy as np
import ml_dtypes
from contextlib import ExitStack

import concourse.bass as bass
import concourse.mybir as mybir
import concourse.tile as tile
from concourse import bacc
from concourse.bass_utils import run_bass_kernel_spmd

BF16 = ml_dtypes.bfloat16
F32 = mybir.dt.float32
BF = mybir.dt.bfloat16
I16 = mybir.dt.int16
AO = mybir.AluOpType
AF = mybir.ActivationFunctionType

B, N, F, E, DEG, H = 4, 10000, 256, 5, 16, 256
NLOC = N // 8          # 1250 nodes per core
NBLK = 10              # 128-node blocks per core (1280 padded)
NPADC = NBLK * 128     # 1280
BLK = 512              # LN/MLP block = 4 batches x 128 nodes
VCOLS = NBLK * BLK     # 5120 virtual columns per core
ZROW = N               # zero-row index in gather table
GIDX = DEG * 128       # 2048 indices per gather call
ROWE = B * F           # 1024 elems per fused row
NQUEUES = int(os.environ.get("KNQ", "1"))
LN_EPS = 1e-6

_PROGRAM = {}
VARIANT = os.environ.get("KVARIANT", "full")  # full | gather | nogather
# build stages for bisection: 1=gather+reduce, 2=+mean, 3=+transpose,
# 4=+Wmatmul+aggevict, 5=full (LN/MLP/stores)
STAGE = int(os.environ.get("KSTAGE", "5"))


def _build_program(repeat=1):
    nc = bacc.Bacc(
        "TRN2",
        target_bir_lowering=False,
        debug=False,
        dynamic_dma_scratch_size=32768,
        num_swdge_queues=NQUEUES,
    )

    tbl4 = nc.dram_tensor("tbl4", [N + 1, ROWE], BF, kind="ExternalInput")
    hT = nc.dram_tensor("hT", [F, VCOLS], F32, kind="ExternalInput")
    idxw = nc.dram_tensor("idxw", [128, E * NBLK * 128], I16, kind="ExternalInput")
    masknm = nc.dram_tensor("masknm", [128, E, NBLK, DEG], F32, kind="ExternalInput")
    w_pe = nc.dram_tensor("w_pe", [128, E * 4, 128], BF, kind="ExternalInput")
    w1 = nc.dram_tensor("w1", [128, 8, 128], BF, kind="ExternalInput")
    w2 = nc.dram_tensor("w2", [128, 4, 128], BF, kind="ExternalInput")
    ident_d = nc.dram_tensor("ident", [128, 128], BF, kind="ExternalInput")
    onesf_d = nc.dram_tensor("onesf", [128, 128], BF, kind="ExternalInput")
    b1_d = nc.dram_tensor("b1pc", [128, 2], F32, kind="ExternalInput")
    b2_d = nc.dram_tensor("b2pc", [128, 2], F32, kind="ExternalInput")
    lns_d = nc.dram_tensor("lnspc", [128, 4], F32, kind="ExternalInput")
    lnb_d = nc.dram_tensor("lnbpc", [128, 4], F32, kind="ExternalInput")
    bedg_d = nc.dram_tensor("bedgpc", [128, 2, E], F32, kind="ExternalInput")

    outT = nc.dram_tensor("outT", [F, VCOLS], F32, kind="ExternalOutput")

    with tile.TileContext(nc) as tc, ExitStack() as ctx:
        cpool = ctx.enter_context(tc.tile_pool(name="const", bufs=1))
        spsum = ctx.enter_context(tc.tile_pool(name="spsum", bufs=2, space="PSUM"))
        mtpsum = ctx.enter_context(tc.tile_pool(name="mtpsum", bufs=2, space="PSUM"))
        apsum = ctx.enter_context(tc.tile_pool(name="apsum", bufs=1, space="PSUM"))
        gpool = ctx.enter_context(tc.tile_pool(name="g", bufs=2))
        xpool = ctx.enter_context(tc.tile_pool(name="x", bufs=2))
        wpool = ctx.enter_context(tc.tile_pool(name="work", bufs=2))

        # ---------------- constants ----------------
        W_sb = cpool.tile([128, E * 4, 128], BF)
        nc.sync.dma_start(W_sb[:], w_pe[:])
        W1_sb = cpool.tile([128, 8, 128], BF)
        nc.sync.dma_start(W1_sb[:], w1[:])
        W2_sb = cpool.tile([128, 4, 128], BF)
        nc.sync.dma_start(W2_sb[:], w2[:])
        id_sb = cpool.tile([128, 128], BF)
        nc.sync.dma_start(id_sb[:], ident_d[:])
        of_sb = cpool.tile([128, 128], BF)
        nc.sync.dma_start(of_sb[:], onesf_d[:])
        b1_sb = cpool.tile([128, 2], F32)
        nc.sync.dma_start(b1_sb[:], b1_d[:])
        b2_sb = cpool.tile([128, 2], F32)
        nc.sync.dma_start(b2_sb[:], b2_d[:])
        lns_sb = cpool.tile([128, 4], F32)
        nc.sync.dma_start(lns_sb[:], lns_d[:])
        lnb_sb = cpool.tile([128, 4], F32)
        nc.sync.dma_start(lnb_sb[:], lnb_d[:])
        bedg_sb = cpool.tile([128, 2, E], F32)
        nc.sync.dma_start(bedg_sb[:], bedg_d[:])
        bsum_sb = cpool.tile([128, 2], F32)
        nc.vector.tensor_reduce(
            bsum_sb[:], bedg_sb[:], axis=mybir.AxisListType.X, op=AO.add
        )

        # all gather indices, resident in SBUF for the whole kernel
        idx_sb = cpool.tile([128, E * NBLK * 128], I16)
        nc.sync.dma_start(idx_sb[:], idxw[:])

        # ---------------- reciprocal degree (node-major) ----------------
        mask_sb = cpool.tile([128, E, NBLK, DEG], F32)
        nc.sync.dma_start(mask_sb[:], masknm[:])
        dn_sb = cpool.tile([128, E, NBLK], F32)
        for i in range(E):
            nc.vector.tensor_reduce(
                dn_sb[:, i, :], mask_sb[:, i], axis=mybir.AxisListType.X, op=AO.add
            )
        nc.vector.tensor_scalar_max(dn_sb[:], dn_sb[:], 1.0)
        recip_sb = cpool.tile([128, E, NBLK], F32)
        nc.vector.reciprocal(recip_sb[:], dn_sb[:])

        def emit_gather_block(kk):
            """Gathers + masked-mean + aggregation for node block kk.
            Returns the assembled x tile [128, 4, BLK]."""
            ns = bass.ts(kk, BLK)
            x = xpool.tile([128, 4, BLK], F32)
            nc.scalar.dma_start(x[:, 0, :], hT[0:128, ns])
            nc.scalar.dma_start(x[:, 1, :], hT[128:256, ns])

            agg = apsum.tile([128, 2, BLK], F32, tag="ps")
            for i in range(E):
                G = gpool.tile([128, DEG, ROWE], BF, tag="G")
                ga = G[:]
                gap = bass.AP(
                    ga.tensor, ga.offset, [ga.ap[0], [ROWE, DEG], [1, ROWE]]
                )
                if VARIANT != "nogather":
                    nc.gpsimd.dma_gather(
                        out_ap=gap,
                        in_ap=tbl4.ap(),
                        idxs_ap=idx_sb[:, (i * NBLK + kk) * 128 : (i * NBLK + kk + 1) * 128],
                        num_idxs=GIDX,
                        num_idxs_reg=GIDX,
                        elem_size=ROWE,
                        single_packet=False,
                        queue_num=(i * NBLK + kk) % NQUEUES,
                    )
                S = spsum.tile([128, 2, BLK], F32, tag="S")
                ND = 1 if VARIANT == "gather" else DEG
                for hh in range(2):
                    for d in range(ND):
                        nc.tensor.matmul(
                            S[:, hh, :],
                            id_sb[:],
                            G[:, d, hh * BLK : (hh + 1) * BLK],
                            start=(d == 0),
                            stop=(d == ND - 1),
                        )
                if VARIANT == "gather" or STAGE < 2:
                    continue
                # masked mean: per-partition (node) reciprocal scale
                mean = wpool.tile([128, 2, BLK], BF, tag="mean")
                for hh in range(2):
                    nc.scalar.activation(
                        mean[:, hh, :], S[:, hh, :], AF.Copy,
                        scale=recip_sb[:, i, kk : kk + 1],
                    )
                if STAGE < 3:
                    continue
                # transpose to feature-major; mean cols: b*256 + c*128 + f
                mT = mtpsum.tile([128, 8, 128], BF, tag="mT")
                mv = mean[:]
                mflat = bass.AP(mv.tensor, mv.offset, [mv.ap[0], [1, 2 * BLK]])
                for bc in range(8):
                    nc.tensor.transpose(
                        mT[:, bc, :],
                        mflat[:, bc * 128 : (bc + 1) * 128],
                        id_sb[:],
                    )
                mT_sb = wpool.tile([128, 8, 128], BF, tag="mTsb")
                nc.scalar.copy(mT_sb[:], mT[:])
                if STAGE < 4:
                    continue
                # aggregated[f, m, b*128+n] += W_i[c,m]^T @ meanT[b,c]
                # (exactly one start=True per PSUM bank: i==0, b==0, c==0)
                for m in range(2):
                    for b in range(4):
                        for c in range(2):
                            nc.tensor.matmul(
                                agg[:, m, b * 128 : (b + 1) * 128],
                                W_sb[:, (i * 2 + c) * 2 + m, :],
                                mT_sb[:, b * 2 + c, :],
                                start=(i == 0 and b == 0 and c == 0),
                                stop=(i == E - 1 and b == 3 and c == 1),
                            )
            # aggregated -> x bottom half (+ sum of edge biases)
            for m in range(2 if (VARIANT != "gather" and STAGE >= 4) else 0):
                nc.scalar.activation(
                    x[:, 2 + m, :],
                    agg[:, m, :],
                    AF.Identity,
                    bias=bsum_sb[:, m : m + 1],
                    scale=1.0,
                )
            return x

        def emit_ln_mlp(kk, x):
            """LayerNorm + MLP + residual + store for block kk given x."""
            ns = bass.ts(kk, BLK)
            st = apsum.tile([128, 2, BLK], F32, tag="ps")
            xbs = []
            for c in range(4):
                xb = wpool.tile([128, BLK], BF, tag=f"xb{c}")
                nc.vector.tensor_copy(xb[:], x[:, c, :])
                xbs.append(xb)
                nc.tensor.matmul(
                    st[:, 0, :], of_sb[:], xb[:],
                    start=(c == 0), stop=(c == 3),
                )
            for c in range(4):
                xsq = wpool.tile([128, BLK], BF, tag="xsq")
                nc.scalar.square(xsq[:], xbs[c][:])
                nc.tensor.matmul(
                    st[:, 1, :], of_sb[:], xsq[:],
                    start=(c == 0), stop=(c == 3),
                )
            # mu/rstd, broadcast across partitions already (ones stationary)
            mu_t = wpool.tile([128, BLK], F32, tag="mu")
            nc.vector.tensor_scalar_mul(mu_t[:], st[:, 0, :], 1.0 / 512.0)
            mu2 = wpool.tile([128, BLK], F32, tag="mu2")
            nc.vector.tensor_mul(mu2[:], mu_t[:], mu_t[:])
            nc.vector.tensor_scalar_sub(mu2[:], mu2[:], LN_EPS)
            var = wpool.tile([128, BLK], F32, tag="var")
            nc.vector.scalar_tensor_tensor(
                var[:], st[:, 1, :], 1.0 / 512.0, mu2[:],
                op0=AO.mult, op1=AO.subtract,
            )
            sd = wpool.tile([128, BLK], F32, tag="sd")
            nc.scalar.activation(sd[:], var[:], AF.Sqrt, bias=0.0)
            rstd = wpool.tile([128, BLK], F32, tag="rstd")
            nc.vector.reciprocal(rstd[:], sd[:])

            xln = wpool.tile([128, 4, BLK], BF, tag="xln")
            for c in range(4):
                tt = wpool.tile([128, BLK], F32, tag="tt")
                nc.vector.scalar_tensor_tensor(
                    tt[:], x[:, c, :], 0.0, mu_t[:],
                    op0=AO.add, op1=AO.subtract,
                )
                nc.vector.tensor_mul(tt[:], tt[:], rstd[:])
                nc.scalar.activation(
                    xln[:, c, :], tt[:], AF.Identity,
                    bias=lnb_sb[:, c : c + 1], scale=lns_sb[:, c : c + 1],
                )

            # ---------------- MLP ----------------
            y1 = apsum.tile([128, 2, BLK], F32, tag="ps")
            for m in range(2):
                for k in range(4):
                    nc.tensor.matmul(
                        y1[:, m, :], W1_sb[:, k * 2 + m, :], xln[:, k, :],
                        start=(k == 0), stop=(k == 3),
                    )
            y1b = wpool.tile([128, 2, BLK], BF, tag="y1b")
            for m in range(2):
                nc.scalar.activation(
                    y1b[:, m, :], y1[:, m, :], AF.Relu,
                    bias=b1_sb[:, m : m + 1], scale=1.0,
                )
            y2 = apsum.tile([128, 2, BLK], F32, tag="ps")
            for m in range(2):
                for k in range(2):
                    nc.tensor.matmul(
                        y2[:, m, :], W2_sb[:, k * 2 + m, :], y1b[:, k, :],
                        start=(k == 0), stop=(k == 1),
                    )
            ot = wpool.tile([128, 2, BLK], F32, tag="ot")
            for m in range(2):
                nc.vector.scalar_tensor_tensor(
                    ot[:, m, :], y2[:, m, :], b2_sb[:, m : m + 1], x[:, m, :],
                    op0=AO.add, op1=AO.add,
                )
            for m in range(2):
                nc.sync.dma_start(outT[m * 128 : (m + 1) * 128, ns], ot[:, m, :])

        # ---------------- software-pipelined main loop ----------------
        do_tail = VARIANT != "gather" and STAGE >= 5
        for rep in range(repeat):
            prev = None  # (kk, x)
            for kk in range(NBLK):
                x = emit_gather_block(kk)
                if prev is not None and do_tail:
                    emit_ln_mlp(*prev)
                prev = (kk, x)
            if do_tail:
                emit_ln_mlp(*prev)

    nc.compile()
    return nc


def _get_program(repeat=1):
    key = (repeat, VARIANT)
    if key not in _PROGRAM:
        _PROGRAM[key] = _build_program(repeat)
    return _PROGRAM[key]


def _prep_shared(h, edge_indices, edge_masks, W_edge, b_edge, ln_scale, ln_bias,
                 W1, b1, W2, b2):
    """Host-side layout prep: weights/constants (shared) + per-core indices."""
    W_pe = np.empty((128, E * 4, 128), np.float32)
    for i in range(E):
        for c in range(2):
            for m in range(2):
                W_pe[:, (i * 2 + c) * 2 + m, :] = W_edge[
                    i, c * 128 : (c + 1) * 128, m * 128 : (m + 1) * 128
                ]
    W1b = np.empty((128, 8, 128), np.float32)
    for k in range(4):
        for m in range(2):
            W1b[:, k * 2 + m, :] = W1[k * 128 : (k + 1) * 128, m * 128 : (m + 1) * 128]
    W2b = np.empty((128, 4, 128), np.float32)
    for k in range(2):
        for m in range(2):
            W2b[:, k * 2 + m, :] = W2[k * 128 : (k + 1) * 128, m * 128 : (m + 1) * 128]

    # 4-batch fused gather table: tbl4[v, b*256+f] = h[b, v, f]
    tbl4 = np.zeros((N + 1, ROWE), BF16)
    tbl4[:N] = np.ascontiguousarray(
        h.transpose(1, 0, 2).reshape(N, ROWE)
    ).astype(BF16)

    shared = dict(
        tbl4=tbl4,
        w_pe=W_pe.astype(BF16),
        w1=W1b.astype(BF16),
        w2=W2b.astype(BF16),
        ident=np.eye(128, dtype=BF16),
        onesf=np.ones((128, 128), BF16),
        b1pc=np.ascontiguousarray(b1.reshape(2, 128).T.astype(np.float32)),
        b2pc=np.ascontiguousarray(b2.reshape(2, 128).T.astype(np.float32)),
        lnspc=np.ascontiguousarray(ln_scale.reshape(4, 128).T.astype(np.float32)),
        lnbpc=np.ascontiguousarray(ln_bias.reshape(4, 128).T.astype(np.float32)),
        bedgpc=np.ascontiguousarray(
            b_edge.T.reshape(2, 128, E).transpose(1, 0, 2).astype(np.float32)
        ),
    )

    cores = []
    for core in range(8):
        n0 = core * NLOC
        idx = edge_indices[:, n0 : n0 + NLOC, :].astype(np.int64)  # [E, NLOC, DEG]
        msk = edge_masks[:, n0 : n0 + NLOC, :]
        idx = np.where(idx < 0, 0, idx)
        sel = np.where(msk > 0, idx, ZROW).astype(np.int32)
        sel = np.concatenate(
            [sel, np.full((E, NPADC - NLOC, DEG), ZROW, np.int32)], axis=1
        )  # [E, NPADC, DEG]
        selT = sel.transpose(0, 2, 1)  # [E, DEG, NPADC]
        blocks = selT.reshape(E, DEG, NBLK, 128).transpose(0, 2, 1, 3)
        L = blocks.reshape(E, NBLK, GIDX)  # j = d*128 + n
        Wv = L.reshape(E, NBLK, GIDX // 16, 16).transpose(0, 1, 3, 2)
        idxw = np.tile(Wv, (1, 1, 8, 1)).astype(np.int16)  # [E, NBLK, 128, 128]
        # SBUF-resident layout: [128, E*NBLK*128]
        idxw = np.ascontiguousarray(
            idxw.transpose(2, 0, 1, 3).reshape(128, E * NBLK * 128)
        )

        mpad = np.concatenate(
            [msk, np.zeros((E, NPADC - NLOC, DEG), np.float32)], axis=1
        )  # [E, NPADC, DEG]
        masknm = np.ascontiguousarray(
            mpad.reshape(E, NBLK, 128, DEG).transpose(2, 0, 1, 3).astype(np.float32)
        )  # [128, E, NBLK, DEG]

        # hT[f, kk*512 + b*128 + n] = h[b, n0 + kk*128 + n, f]
        hp = np.zeros((B, NPADC, F), np.float32)
        hp[:, :NLOC] = h[:, n0 : n0 + NLOC, :]
        hTl = np.ascontiguousarray(
            hp.reshape(B, NBLK, 128, F).transpose(3, 1, 0, 2).reshape(F, VCOLS)
        )
        m = dict(hT=hTl, idxw=idxw, masknm=masknm)
        m.update(shared)
        cores.append(m)
    return cores


def kernel(**inputs):
    h = np.asarray(inputs["h"], np.float32)
    nc = _get_program()
    in_maps = _prep_shared(
        h,
        np.asarray(inputs["edge_indices"]),
        np.asarray(inputs["edge_masks"], np.float32),
        np.asarray(inputs["W_edge"], np.float32),
        np.asarray(inputs["b_edge"], np.float32),
        np.asarray(inputs["ln_scale"], np.float32),
        np.asarray(inputs["ln_bias"], np.float32),
        np.asarray(inputs["W1"], np.float32),
        np.asarray(inputs["b1"], np.float32),
        np.asarray(inputs["W2"], np.float32),
        np.asarray(inputs["b2"], np.float32),
    )

    res = run_bass_kernel_spmd(nc, in_maps, core_ids=list(range(8)))

    out = np.empty((B, N, F), np.float32)
    for core in range(8):
        n0 = core * NLOC
        o = res.results[core]["outT"]  # [F, VCOLS]
        ob = o.reshape(F, NBLK, B, 128).transpose(2, 1, 3, 0).reshape(B, NPADC, F)
        out[:, n0 : n0 + NLOC, :] = ob[:, :NLOC]
    return out

